# revision 2
# baseline (speedup 1.0000x reference)
"""Trainium2 Bass kernel for nn_DfDecoderStep (GRU decoder step + causal conv).

Strategy: pure data-parallel over the batch dim across 8 NeuronCores
(512 rows/core).  On-chip everything is computed feature-major
(features on SBUF partitions, batch in the free dim) so every matmul
contracts along partitions with the weights as the stationary operand.
Inputs are transposed on-chip via PE transposes (fp32 DMA transpose is
unsupported); the conv input transposes dominate and are pipelined with
the conv matmuls.  The grouped linears / grouped conv / pointwise conv
are expressed as dense block-diagonal lhsT matrices precomputed on the
host (they are tiny).
"""

import numpy as np

import concourse.bacc as bacc
import concourse.tile as tile
from concourse import mybir
from concourse.bass_utils import run_bass_kernel_spmd

B, EMB, H, G, F, CH, O, OC, KT = 4096, 256, 256, 8, 96, 16, 5, 10, 5
BN_EPS = 1e-3
NCORES = 8
BS = B // NCORES          # 512 rows per core
NBC = BS // 128           # 4 batch chunks of 128
FCIN = F * CH             # 1536 features per conv frame
NFB = 12                  # conv feature blocks (8 f-values x 10 (g,o) = 80 outs each)

f32 = mybir.dt.float32
AF = mybir.ActivationFunctionType
OP = mybir.AluOpType

# which 128-wide cT tile each w_out output block contracts against
TAU = [(32 * ((80 * fb) // 120)) // 128 for fb in range(NFB)]


def _build_nc():
    nc = bacc.Bacc("TRN2")

    # ---- I/O ----
    emb_d = nc.dram_tensor("emb", [BS, EMB], f32, kind="ExternalInput")
    st_d = nc.dram_tensor("state", [BS, 2 * H], f32, kind="ExternalInput")
    buf_d = nc.dram_tensor("buf", [BS, KT - 1, F, CH], f32, kind="ExternalInput")
    c0_d = nc.dram_tensor("c0", [BS, 1, F, CH], f32, kind="ExternalInput")
    winl_d = nc.dram_tensor("winl", [2, 128, 128], f32, kind="ExternalInput")
    k0_d = nc.dram_tensor("k0", [H, 3 * H], f32, kind="ExternalInput")
    rk0_d = nc.dram_tensor("rk0", [H, 3 * H], f32, kind="ExternalInput")
    k1_d = nc.dram_tensor("k1", [H, 3 * H], f32, kind="ExternalInput")
    rk1_d = nc.dram_tensor("rk1", [H, 3 * H], f32, kind="ExternalInput")
    bz0_d = nc.dram_tensor("bz0", [4, 128, 1], f32, kind="ExternalInput")
    bxh0_d = nc.dram_tensor("bxh0", [2, 128, 1], f32, kind="ExternalInput")
    bhh0_d = nc.dram_tensor("bhh0", [2, 128, 1], f32, kind="ExternalInput")
    bz1_d = nc.dram_tensor("bz1", [4, 128, 1], f32, kind="ExternalInput")
    bxh1_d = nc.dram_tensor("bxh1", [2, 128, 1], f32, kind="ExternalInput")
    bhh1_d = nc.dram_tensor("bhh1", [2, 128, 1], f32, kind="ExternalInput")
    woutl_d = nc.dram_tensor("woutl", [NFB, 128, 80], f32, kind="ExternalInput")
    convl_d = nc.dram_tensor("convl", [KT, 128, 80], f32, kind="ExternalInput")
    pwl_d = nc.dram_tensor("pwl", [80, 80], f32, kind="ExternalInput")
    bns_d = nc.dram_tensor("bns", [80, 1], f32, kind="ExternalInput")
    bnb_d = nc.dram_tensor("bnb", [80, 1], f32, kind="ExternalInput")
    fcw_d = nc.dram_tensor("fcw", [2, 128, 1], f32, kind="ExternalInput")
    fcb_d = nc.dram_tensor("fcb", [1, 1], f32, kind="ExternalInput")
    id_d = nc.dram_tensor("ident", [128, 128], f32, kind="ExternalInput")

    coefs_d = nc.dram_tensor("coefs", [BS, F * OC], f32, kind="ExternalOutput")
    alpha_d = nc.dram_tensor("alpha", [BS, 1], f32, kind="ExternalOutput")
    stout_d = nc.dram_tensor("state_out", [BS, 2 * H], f32, kind="ExternalOutput")
    bufout_d = nc.dram_tensor(
        "buf_out", [BS, KT - 1, F, CH], f32, kind="ExternalOutput"
    )

    with tile.TileContext(nc) as tc:
        with (
            tc.tile_pool(name="singles", bufs=1) as singles,
            tc.tile_pool(name="gwp", bufs=2) as gwp,
            tc.tile_pool(name="featp", bufs=1) as featp,
            tc.tile_pool(name="bm", bufs=4) as bmp,
            tc.tile_pool(name="gwork", bufs=10) as gwork,
            tc.tile_pool(name="xp", bufs=4) as xp,
            tc.tile_pool(name="xtp", bufs=66) as xtp,
            tc.tile_pool(name="thp", bufs=12) as thp,
            tc.tile_pool(name="yp", bufs=4) as yp,
            tc.tile_pool(name="cbp", bufs=2) as cbp,
            tc.tile_pool(name="ps", bufs=6, space="PSUM") as psp,
            tc.tile_pool(name="ptr", bufs=2, space="PSUM") as ptr,
        ):
            # ---------- constants / weights ----------
            ident = singles.tile([128, 128], f32, tag="ident")
            nc.sync.dma_start(ident, id_d[:, :])
            winl = singles.tile([128, 2, 128], f32, tag="winl")
            for c in range(2):
                nc.sync.dma_start(winl[:, c, :], winl_d[c])
            woutl = singles.tile([128, NFB, 80], f32, tag="woutl")
            for j in range(NFB):
                nc.sync.dma_start(woutl[:, j, :], woutl_d[j])
            convl = singles.tile([128, KT, 80], f32, tag="convl")
            for t in range(KT):
                nc.sync.dma_start(convl[:, t, :], convl_d[t])
            pwl = singles.tile([80, 80], f32, tag="pwl")
            nc.sync.dma_start(pwl, pwl_d[:, :])
            bns = singles.tile([80, 1], f32, tag="bns")
            nc.sync.dma_start(bns, bns_d[:, :])
            bnb = singles.tile([80, 1], f32, tag="bnb")
            nc.sync.dma_start(bnb, bnb_d[:, :])
            fcw = singles.tile([128, 2], f32, tag="fcw")
            for c in range(2):
                nc.sync.dma_start(fcw[:, c : c + 1], fcw_d[c])
            fcb = singles.tile([1, 1], f32, tag="fcb")
            nc.sync.dma_start(fcb, fcb_d[:, :])

            def bias_tile(dram, n, tag):
                t = singles.tile([128, n], f32, tag=tag)
                for c in range(n):
                    nc.sync.dma_start(t[:, c : c + 1], dram[c])
                return t

            bz0 = bias_tile(bz0_d, 4, "bz0")
            bxh0 = bias_tile(bxh0_d, 2, "bxh0")
            bhh0 = bias_tile(bhh0_d, 2, "bhh0")
            bz1 = bias_tile(bz1_d, 4, "bz1")
            bxh1 = bias_tile(bxh1_d, 2, "bxh1")
            bhh1 = bias_tile(bhh1_d, 2, "bhh1")

            # ---------- persistent feature-major activations ----------
            embT = featp.tile([128, 2, BS], f32, tag="embT")
            xinT = featp.tile([128, 2, BS], f32, tag="xinT")
            h0T = featp.tile([128, 2, BS], f32, tag="h0T")
            h1T = featp.tile([128, 2, BS], f32, tag="h1T")
            o0T = featp.tile([128, 2, BS], f32, tag="o0T")
            o1T = featp.tile([128, 2, BS], f32, tag="o1T")
            cT = featp.tile([128, 2, BS], f32, tag="cT")

            # ---------- load + transpose emb and state ----------
            for bc in range(NBC):
                bsl = slice(bc * 128, (bc + 1) * 128)
                ebm = bmp.tile([128, EMB], f32, tag="ebm")
                nc.sync.dma_start(ebm, emb_d[bsl, :])
                for c in range(2):
                    pt = ptr.tile([128, 128], f32, tag="pt")
                    nc.tensor.transpose(pt, ebm[:, c * 128 : (c + 1) * 128], ident)
                    nc.vector.tensor_copy(embT[:, c, bsl], pt)
                sbm = bmp.tile([128, 2 * H], f32, tag="sbm")
                nc.sync.dma_start(sbm, st_d[bsl, :])
                for tau in range(4):
                    pt = ptr.tile([128, 128], f32, tag="pt")
                    nc.tensor.transpose(
                        pt, sbm[:, tau * 128 : (tau + 1) * 128], ident
                    )
                    dst = h0T if tau < 2 else h1T
                    nc.vector.tensor_copy(dst[:, tau % 2, bsl], pt)

            # ---------- in-projection + relu ----------
            for c in range(2):
                ps = psp.tile([128, BS], f32, tag="ps")
                nc.tensor.matmul(
                    ps, winl[:, c, :], embT[:, c, :], start=True, stop=True
                )
                nc.vector.tensor_scalar_max(xinT[:, c, :], ps, 0.0)

            # ---------- GRU cells ----------
            def gru_cell(xT, hT, kd, rkd, bz, bxh, bhh, outT):
                gk = gwp.tile([128, 2, 3 * H], f32, tag="gw")
                gr = gwp.tile([128, 2, 3 * H], f32, tag="gw")
                for kc in range(2):
                    nc.sync.dma_start(gk[:, kc, :], kd[kc * 128 : (kc + 1) * 128, :])
                    nc.sync.dma_start(
                        gr[:, kc, :], rkd[kc * 128 : (kc + 1) * 128, :]
                    )
                zs, rs = [], []
                for c in range(4):  # z gates: c 0,1; r gates: c 2,3
                    ps = psp.tile([128, BS], f32, tag="ps")
                    msl = slice(c * 128, (c + 1) * 128)
                    nc.tensor.matmul(
                        ps, gk[:, 0, msl], xT[:, 0, :], start=True, stop=False
                    )
                    nc.tensor.matmul(
                        ps, gk[:, 1, msl], xT[:, 1, :], start=False, stop=False
                    )
                    nc.tensor.matmul(
                        ps, gr[:, 0, msl], hT[:, 0, :], start=False, stop=False
                    )
                    nc.tensor.matmul(
                        ps, gr[:, 1, msl], hT[:, 1, :], start=False, stop=True
                    )
                    g = gwork.tile([128, BS], f32, tag="gt")
                    nc.scalar.activation(
                        g, ps, AF.Sigmoid, bias=bz[:, c : c + 1], scale=1.0
                    )
                    (zs if c < 2 else rs).append(g)
                for c in range(2):  # candidate + new h
                    msl = slice((4 + c) * 128, (5 + c) * 128)
                    psx = psp.tile([128, BS], f32, tag="ps")
                    nc.tensor.matmul(
                        psx, gk[:, 0, msl], xT[:, 0, :], start=True, stop=False
                    )
                    nc.tensor.matmul(
                        psx, gk[:, 1, msl], xT[:, 1, :], start=False, stop=True
                    )
                    psh = psp.tile([128, BS], f32, tag="ps")
                    nc.tensor.matmul(
                        psh, gr[:, 0, msl], hT[:, 0, :], start=True, stop=False
                    )
                    nc.tensor.matmul(
                        psh, gr[:, 1, msl], hT[:, 1, :], start=False, stop=True
                    )
                    t1 = gwork.tile([128, BS], f32, tag="gt")
                    nc.vector.scalar_tensor_tensor(
                        t1, psh, bhh[:, c : c + 1], rs[c], OP.add, OP.mult
                    )
                    t3 = gwork.tile([128, BS], f32, tag="gt")
                    nc.vector.tensor_add(t3, psx, t1)
                    cand = gwork.tile([128, BS], f32, tag="gt")
                    nc.scalar.activation(
                        cand, t3, AF.Tanh, bias=bxh[:, c : c + 1], scale=1.0
                    )
                    d = gwork.tile([128, BS], f32, tag="gt")
                    nc.vector.tensor_sub(d, hT[:, c, :], cand)
                    e = gwork.tile([128, BS], f32, tag="gt")
                    nc.vector.tensor_mul(e, d, zs[c])
                    nc.vector.tensor_add(outT[:, c, :], e, cand)

            gru_cell(xinT, h0T, k0_d, rk0_d, bz0, bxh0, bhh0, o0T)
            gru_cell(o0T, h1T, k1_d, rk1_d, bz1, bxh1, bhh1, o1T)

            # skip connection
            for c in range(2):
                nc.vector.tensor_add(cT[:, c, :], o1T[:, c, :], xinT[:, c, :])

            # ---------- new_state output ----------
            for bc in range(NBC):
                bsl = slice(bc * 128, (bc + 1) * 128)
                so = bmp.tile([128, 2 * H], f32, tag="so")
                for c in range(2):
                    pt = ptr.tile([128, 128], f32, tag="pt")
                    nc.tensor.transpose(pt, o0T[:, c, bsl], ident)
                    nc.vector.tensor_copy(so[:, c * 128 : (c + 1) * 128], pt)
                    pt2 = ptr.tile([128, 128], f32, tag="pt")
                    nc.tensor.transpose(pt2, o1T[:, c, bsl], ident)
                    nc.vector.tensor_copy(
                        so[:, 256 + c * 128 : 256 + (c + 1) * 128], pt2
                    )
                nc.sync.dma_start(stout_d[bsl, :], so)

            # ---------- alpha ----------
            psa = psp.tile([1, BS], f32, tag="ps")
            nc.tensor.matmul(psa, fcw[:, 0:1], cT[:, 0, :], start=True, stop=False)
            nc.tensor.matmul(psa, fcw[:, 1:2], cT[:, 1, :], start=False, stop=True)
            al = singles.tile([1, BS], f32, tag="alpha")
            nc.scalar.activation(al, psa, AF.Sigmoid, bias=fcb[0:1, 0:1], scale=1.0)
            nc.sync.dma_start(alpha_d.rearrange("b one -> one b"), al)

            # ---------- w_out grouped linear + tanh ----------
            th = []
            for fb in range(NFB):
                ps = psp.tile([80, BS], f32, tag="ps")
                nc.tensor.matmul(
                    ps, woutl[:, fb, :], cT[:, TAU[fb], :], start=True, stop=True
                )
                t = thp.tile([80, BS], f32, tag="th")
                nc.scalar.activation(t, ps, AF.Tanh, scale=1.0)
                th.append(t)

            # ---------- conv path ----------
            for bc in range(NBC):
                bsl = slice(bc * 128, (bc + 1) * 128)
                xts = {}
                for t in range(KT):
                    xs = xp.tile([128, FCIN], f32, tag="xs")
                    if t < KT - 1:
                        nc.sync.dma_start(
                            xs, buf_d[bsl, t].rearrange("b f c -> b (f c)")
                        )
                    else:
                        nc.sync.dma_start(
                            xs, c0_d[bsl, 0].rearrange("b f c -> b (f c)")
                        )
                    if t >= 1:
                        nc.sync.dma_start(
                            bufout_d[bsl, t - 1].rearrange("b f c -> b (f c)"), xs
                        )
                    for fb in range(NFB):
                        pt = ptr.tile([128, 128], f32, tag="pt")
                        nc.tensor.transpose(
                            pt, xs[:, fb * 128 : (fb + 1) * 128], ident
                        )
                        xt = xtp.tile([128, 128], f32, tag="xt")
                        nc.vector.tensor_copy(xt, pt)
                        xts[(t, fb)] = xt

                cb = cbp.tile([128, F * OC], f32, tag="cb")
                cbv = cb.rearrange("p (o fr t) -> p fr o t", o=O, t=2)
                for fbg in range(2):
                    pcs = [
                        psp.tile([80, 128], f32, tag="ps", name=f"pcs{fbg}_{j}")
                        for j in range(6)
                    ]
                    for t in range(KT):
                        for j in range(6):
                            nc.tensor.matmul(
                                pcs[j],
                                convl[:, t, :],
                                xts[(t, fbg * 6 + j)],
                                start=(t == 0),
                                stop=(t == KT - 1),
                            )
                    for j in range(6):
                        fb = fbg * 6 + j
                        ysl = yp.tile([80, 128], f32, tag="ys")
                        nc.vector.tensor_copy(ysl, pcs[j])
                        pp = psp.tile([80, 128], f32, tag="ps")
                        nc.tensor.matmul(pp, pwl, ysl, start=True, stop=True)
                        c0o = yp.tile([80, 128], f32, tag="c0o")
                        nc.scalar.activation(
                            c0o, pp, AF.Relu, bias=bnb[:, 0:1], scale=bns[:, 0:1]
                        )
                        co = yp.tile([80, 128], f32, tag="co")
                        nc.vector.tensor_add(co, c0o, th[fb][:, bsl])
                        pt = ptr.tile([128, 80], f32, tag="pt")
                        nc.tensor.transpose(pt, co, ident[:80, :80])
                        nc.vector.tensor_copy(
                            cbv[:, fb * 8 : (fb + 1) * 8, :, :],
                            pt.rearrange("p (fr o t) -> p fr o t", o=O, t=2),
                        )
                nc.sync.dma_start(coefs_d[bsl, :], cb)

    nc.finalize()
    return nc


_NC_CACHE = None


def _get_nc():
    global _NC_CACHE
    if _NC_CACHE is None:
        _NC_CACHE = _build_nc()
    return _NC_CACHE


def _prep_shared(w_in, gru_k0, gru_rk0, gru_b0, gru_k1, gru_rk1, gru_b1,
                 w_out, fc_a_w, fc_a_b, conv_w, pw_w,
                 bn_gamma, bn_beta, bn_mean, bn_var):
    f = np.float32
    shared = {}

    # dense block-diagonal in-projection lhsT: [2, 128, 128]
    winl = np.zeros((2, 128, 128), f)
    for g in range(G):
        c, gg = divmod(g, 4)
        winl[c, gg * 32 : (gg + 1) * 32, gg * 32 : (gg + 1) * 32] = w_in[g]
    shared["winl"] = winl

    shared["k0"] = np.ascontiguousarray(gru_k0, f)
    shared["rk0"] = np.ascontiguousarray(gru_rk0, f)
    shared["k1"] = np.ascontiguousarray(gru_k1, f)
    shared["rk1"] = np.ascontiguousarray(gru_rk1, f)

    for nm, b in (("0", gru_b0), ("1", gru_b1)):
        bz = (b[0, : 2 * H] + b[1, : 2 * H]).astype(f)
        shared["bz" + nm] = bz.reshape(4, 128, 1)
        shared["bxh" + nm] = b[0, 2 * H :].astype(f).reshape(2, 128, 1)
        shared["bhh" + nm] = b[1, 2 * H :].astype(f).reshape(2, 128, 1)

    # w_out grouped-linear as per-feature-block lhsT: [12, 128, 80]
    woutl = np.zeros((NFB, 128, 80), f)
    for fb in range(NFB):
        for j in range(80):
            feat = 80 * fb + j
            gw, og = divmod(feat, 120)
            r0 = 32 * gw - 128 * TAU[fb]
            woutl[fb, r0 : r0 + 32, j] = w_out[gw, :, og]
    shared["woutl"] = woutl

    # conv lhsT per time step: [5, 128, 80]
    wg = conv_w.reshape(KT, CH // 2, 2, OC // 2)  # [t, i, g, o]
    convl = np.zeros((KT, 128, 80), f)
    for t in range(KT):
        for fs in range(8):
            for g in range(2):
                for i in range(8):
                    k = fs * 16 + g * 8 + i
                    for o in range(OC // 2):
                        convl[t, k, fs * 10 + g * 5 + o] = wg[t, i, g, o]
    shared["convl"] = convl

    # pointwise conv lhsT: block-diag of pw_w over 8 f values
    pwl = np.zeros((80, 80), f)
    for fs in range(8):
        pwl[fs * 10 : (fs + 1) * 10, fs * 10 : (fs + 1) * 10] = pw_w
    shared["pwl"] = pwl

    inv = 1.0 / np.sqrt(bn_var.astype(np.float64) + BN_EPS)
    s10 = (inv * bn_gamma).astype(f)
    b10 = (bn_beta - bn_mean * inv * bn_gamma).astype(f)
    shared["bns"] = np.tile(s10, 8).reshape(80, 1)
    shared["bnb"] = np.tile(b10, 8).reshape(80, 1)

    shared["fcw"] = np.ascontiguousarray(fc_a_w, f).reshape(2, 128, 1)
    shared["fcb"] = np.asarray(fc_a_b, f).reshape(1, 1)
    shared["ident"] = np.eye(128, dtype=f)
    return shared


def kernel(emb, c0, df_dec_state, df_convp_buf,
           w_in, gru_k0, gru_rk0, gru_b0, gru_k1, gru_rk1, gru_b1,
           w_out, fc_a_w, fc_a_b, conv_w, pw_w,
           bn_gamma, bn_beta, bn_mean, bn_var):
    nc = _get_nc()
    shared = _prep_shared(
        np.asarray(w_in, np.float32), np.asarray(gru_k0, np.float32),
        np.asarray(gru_rk0, np.float32), np.asarray(gru_b0, np.float32),
        np.asarray(gru_k1, np.float32), np.asarray(gru_rk1, np.float32),
        np.asarray(gru_b1, np.float32), np.asarray(w_out, np.float32),
        np.asarray(fc_a_w, np.float32), np.asarray(fc_a_b, np.float32),
        np.asarray(conv_w, np.float32), np.asarray(pw_w, np.float32),
        np.asarray(bn_gamma, np.float32), np.asarray(bn_beta, np.float32),
        np.asarray(bn_mean, np.float32), np.asarray(bn_var, np.float32),
    )
    emb = np.ascontiguousarray(emb, np.float32)
    c0 = np.ascontiguousarray(c0, np.float32)
    st = np.ascontiguousarray(df_dec_state, np.float32)
    buf = np.ascontiguousarray(df_convp_buf, np.float32)

    in_maps = []
    for i in range(NCORES):
        sl = slice(i * BS, (i + 1) * BS)
        m = dict(shared)
        m["emb"] = emb[sl]
        m["c0"] = c0[sl]
        m["state"] = st[sl]
        m["buf"] = buf[sl]
        in_maps.append(m)

    res = run_bass_kernel_spmd(nc, in_maps, list(range(NCORES))).results

    coefs = np.concatenate([r["coefs"] for r in res], 0).reshape(B, O, 1, F, 2)
    alpha = np.concatenate([r["alpha"] for r in res], 0)
    new_state = np.concatenate([r["state_out"] for r in res], 0)
    new_buf = np.concatenate([r["buf_out"] for r in res], 0)
    return coefs, alpha, new_state, new_buf


# revision 4
# speedup vs baseline: 1.3735x; 1.3735x over previous
"""Trainium2 Bass kernel for nn_DfDecoderStep (GRU decoder step + causal conv).

Data-parallel over batch across 8 NeuronCores (512 rows/core).  On-chip
compute is feature-major (features on SBUF partitions, batch in the free
dim) so every matmul contracts along partitions with weights stationary.
Inputs are transposed on-chip with PE transposes (fp32, exact); matmuls
run in float32r (single-pass, full-rate at free-dim >= 256; ~tf32
precision, rel err ~1e-4).  Grouped linears / grouped conv / pointwise
conv are dense block-diagonal lhsT matrices precomputed on the host.
The conv buffer shift (new_buf) is an exact fp32 copy.
"""

import numpy as np

import concourse.bacc as bacc
import concourse.tile as tile
from concourse import mybir
from concourse.bass_utils import run_bass_kernel_spmd

B, EMB, H, G, F, CH, O, OC, KT = 4096, 256, 256, 8, 96, 16, 5, 10, 5
BN_EPS = 1e-3
NCORES = 8
BS = B // NCORES          # 512 rows per core
NBC = BS // 128           # 4 batch chunks of 128
FCIN = F * CH             # 1536 features per conv frame
NFB = 12                  # conv feature blocks (8 f x 10 (g,o) = 80 outputs each)
NGRP = 3                  # conv fb groups of 4 (PSUM budget)
NH = 2                    # batch halves (free dim 256 for conv-side matmuls)

f32 = mybir.dt.float32
f32r = mybir.dt.float32r
AF = mybir.ActivationFunctionType
OP = mybir.AluOpType

# which 128-wide cT tile each w_out output block contracts against
TAU = [(32 * ((80 * fb) // 120)) // 128 for fb in range(NFB)]


def _build_nc():
    nc = bacc.Bacc("TRN2")

    # ---- I/O ----
    emb_d = nc.dram_tensor("emb", [BS, EMB], f32, kind="ExternalInput")
    st_d = nc.dram_tensor("state", [BS, 2 * H], f32, kind="ExternalInput")
    buf_d = nc.dram_tensor("buf", [BS, KT - 1, F, CH], f32, kind="ExternalInput")
    c0_d = nc.dram_tensor("c0", [BS, 1, F, CH], f32, kind="ExternalInput")
    winl_d = nc.dram_tensor("winl", [2, 128, 128], f32r, kind="ExternalInput")
    k0_d = nc.dram_tensor("k0", [H, 3 * H], f32r, kind="ExternalInput")
    rk0_d = nc.dram_tensor("rk0", [H, 3 * H], f32r, kind="ExternalInput")
    k1_d = nc.dram_tensor("k1", [H, 3 * H], f32r, kind="ExternalInput")
    rk1_d = nc.dram_tensor("rk1", [H, 3 * H], f32r, kind="ExternalInput")
    bz0_d = nc.dram_tensor("bz0", [4, 128, 1], f32, kind="ExternalInput")
    bxh0_d = nc.dram_tensor("bxh0", [2, 128, 1], f32, kind="ExternalInput")
    bhh0_d = nc.dram_tensor("bhh0", [2, 128, 1], f32, kind="ExternalInput")
    bz1_d = nc.dram_tensor("bz1", [4, 128, 1], f32, kind="ExternalInput")
    bxh1_d = nc.dram_tensor("bxh1", [2, 128, 1], f32, kind="ExternalInput")
    bhh1_d = nc.dram_tensor("bhh1", [2, 128, 1], f32, kind="ExternalInput")
    woutl_d = nc.dram_tensor("woutl", [NFB, 128, 80], f32r, kind="ExternalInput")
    convl_d = nc.dram_tensor("convl", [KT, 128, 80], f32r, kind="ExternalInput")
    pwl_d = nc.dram_tensor("pwl", [80, 80], f32r, kind="ExternalInput")
    bns_d = nc.dram_tensor("bns", [80, 1], f32, kind="ExternalInput")
    bnb_d = nc.dram_tensor("bnb", [80, 1], f32, kind="ExternalInput")
    fcw_d = nc.dram_tensor("fcw", [2, 128, 1], f32r, kind="ExternalInput")
    fcb_d = nc.dram_tensor("fcb", [1, 1], f32, kind="ExternalInput")
    id_d = nc.dram_tensor("ident", [128, 128], f32, kind="ExternalInput")

    coefs_d = nc.dram_tensor("coefs", [BS, F * OC], f32, kind="ExternalOutput")
    alpha_d = nc.dram_tensor("alpha", [BS, 1], f32, kind="ExternalOutput")
    stout_d = nc.dram_tensor("state_out", [BS, 2 * H], f32, kind="ExternalOutput")
    bufout_d = nc.dram_tensor(
        "buf_out", [BS, KT - 1, F, CH], f32, kind="ExternalOutput"
    )

    with tile.TileContext(nc) as tc:
        with (
            tc.tile_pool(name="singles", bufs=1) as singles,
            tc.tile_pool(name="gwp", bufs=2) as gwp,
            tc.tile_pool(name="featp", bufs=1) as featp,
            tc.tile_pool(name="bm", bufs=4) as bmp,
            tc.tile_pool(name="gwork", bufs=10) as gwork,
            tc.tile_pool(name="xg", bufs=4) as xgp,
            tc.tile_pool(name="xtp", bufs=30) as xtp,
            tc.tile_pool(name="wk", bufs=4) as wk,
            tc.tile_pool(name="cbp", bufs=3) as cbp,
            tc.tile_pool(name="ps", bufs=6, space="PSUM") as psp,
            tc.tile_pool(name="ptr", bufs=2, space="PSUM") as ptr,
        ):
            ncopy = 0

            def copy_out(dst, src):
                # alternate PSUM->SBUF copies between DVE and ACT
                nonlocal ncopy
                ncopy += 1
                if ncopy % 2:
                    nc.vector.tensor_copy(dst, src)
                else:
                    nc.scalar.copy(dst, src)

            # ---------- constants / weights ----------
            ident = singles.tile([128, 128], f32, tag="ident")
            nc.sync.dma_start(ident, id_d[:, :])
            winl = singles.tile([128, 2, 128], f32r, tag="winl")
            for c in range(2):
                nc.sync.dma_start(winl[:, c, :], winl_d[c])
            woutl = singles.tile([128, NFB, 80], f32r, tag="woutl")
            for j in range(NFB):
                nc.sync.dma_start(woutl[:, j, :], woutl_d[j])
            convl = singles.tile([128, KT, 80], f32r, tag="convl")
            for t in range(KT):
                nc.sync.dma_start(convl[:, t, :], convl_d[t])
            pwl = singles.tile([80, 80], f32r, tag="pwl")
            nc.sync.dma_start(pwl, pwl_d[:, :])
            bns = singles.tile([80, 1], f32, tag="bns")
            nc.sync.dma_start(bns, bns_d[:, :])
            bnb = singles.tile([80, 1], f32, tag="bnb")
            nc.sync.dma_start(bnb, bnb_d[:, :])
            fcw = singles.tile([128, 2], f32r, tag="fcw")
            for c in range(2):
                nc.sync.dma_start(fcw[:, c : c + 1], fcw_d[c])
            fcb = singles.tile([1, 1], f32, tag="fcb")
            nc.sync.dma_start(fcb, fcb_d[:, :])

            def bias_tile(dram, n, tag):
                t = singles.tile([128, n], f32, tag=tag, name=tag)
                for c in range(n):
                    nc.sync.dma_start(t[:, c : c + 1], dram[c])
                return t

            bz0 = bias_tile(bz0_d, 4, "bz0")
            bxh0 = bias_tile(bxh0_d, 2, "bxh0")
            bhh0 = bias_tile(bhh0_d, 2, "bhh0")
            bz1 = bias_tile(bz1_d, 4, "bz1")
            bxh1 = bias_tile(bxh1_d, 2, "bxh1")
            bhh1 = bias_tile(bhh1_d, 2, "bhh1")

            # ---------- persistent feature-major activations (fp32r) ----------
            embT = featp.tile([128, 2, BS], f32r, tag="embT")
            xinT = featp.tile([128, 2, BS], f32r, tag="xinT")
            h0T = featp.tile([128, 2, BS], f32r, tag="h0T")
            h1T = featp.tile([128, 2, BS], f32r, tag="h1T")
            o0T = featp.tile([128, 2, BS], f32r, tag="o0T")
            o1T = featp.tile([128, 2, BS], f32r, tag="o1T")
            cT = featp.tile([128, 2, BS], f32r, tag="cT")

            # ---------- load + transpose emb and state ----------
            ebms = []
            sbms = []
            for bc in range(NBC):
                bsl = slice(bc * 128, (bc + 1) * 128)
                ebm = bmp.tile([128, EMB], f32, tag="ebm", name=f"ebm{bc}")
                nc.sync.dma_start(ebm, emb_d[bsl, :])
                ebms.append(ebm)
                sbm = bmp.tile([128, 2 * H], f32, tag="sbm", name=f"sbm{bc}")
                nc.sync.dma_start(sbm, st_d[bsl, :])
                sbms.append(sbm)
            for c in range(2):
                pt = ptr.tile([128, BS], f32, tag="pt", name=f"ptemb{c}")
                for bc in range(NBC):
                    nc.tensor.transpose(
                        pt[:, bc * 128 : (bc + 1) * 128],
                        ebms[bc][:, c * 128 : (c + 1) * 128],
                        ident,
                    )
                copy_out(embT[:, c, :], pt)
            for tau in range(4):
                pt = ptr.tile([128, BS], f32, tag="pt", name=f"ptst{tau}")
                for bc in range(NBC):
                    nc.tensor.transpose(
                        pt[:, bc * 128 : (bc + 1) * 128],
                        sbms[bc][:, tau * 128 : (tau + 1) * 128],
                        ident,
                    )
                dst = h0T if tau < 2 else h1T
                copy_out(dst[:, tau % 2, :], pt)

            # ---------- in-projection + relu ----------
            for c in range(2):
                ps = psp.tile([128, BS], f32, tag="ps", name=f"psin{c}")
                nc.tensor.matmul(
                    ps, winl[:, c, :], embT[:, c, :], start=True, stop=True
                )
                nc.vector.tensor_scalar_max(xinT[:, c, :], ps, 0.0)

            # ---------- GRU cells ----------
            def gru_cell(idx, xT, hT, kd, rkd, bz, bxh, bhh, outT):
                gk = gwp.tile([128, 2, 3 * H], f32r, tag="gw", name=f"gk{idx}")
                gr = gwp.tile([128, 2, 3 * H], f32r, tag="gw", name=f"gr{idx}")
                for kc in range(2):
                    nc.sync.dma_start(gk[:, kc, :], kd[kc * 128 : (kc + 1) * 128, :])
                    nc.sync.dma_start(
                        gr[:, kc, :], rkd[kc * 128 : (kc + 1) * 128, :]
                    )
                zs, rs = [], []
                for c in range(4):  # z gates: c 0,1; r gates: c 2,3
                    ps = psp.tile([128, BS], f32, tag="ps", name=f"pszr{idx}{c}")
                    msl = slice(c * 128, (c + 1) * 128)
                    nc.tensor.matmul(
                        ps, gk[:, 0, msl], xT[:, 0, :], start=True, stop=False
                    )
                    nc.tensor.matmul(
                        ps, gk[:, 1, msl], xT[:, 1, :], start=False, stop=False
                    )
                    nc.tensor.matmul(
                        ps, gr[:, 0, msl], hT[:, 0, :], start=False, stop=False
                    )
                    nc.tensor.matmul(
                        ps, gr[:, 1, msl], hT[:, 1, :], start=False, stop=True
                    )
                    g = gwork.tile([128, BS], f32, tag="gt", name=f"g{idx}{c}")
                    nc.scalar.activation(
                        g, ps, AF.Sigmoid, bias=bz[:, c : c + 1], scale=1.0
                    )
                    (zs if c < 2 else rs).append(g)
                for c in range(2):  # candidate + new h
                    msl = slice((4 + c) * 128, (5 + c) * 128)
                    psx = psp.tile([128, BS], f32, tag="ps", name=f"psx{idx}{c}")
                    nc.tensor.matmul(
                        psx, gk[:, 0, msl], xT[:, 0, :], start=True, stop=False
                    )
                    nc.tensor.matmul(
                        psx, gk[:, 1, msl], xT[:, 1, :], start=False, stop=True
                    )
                    psh = psp.tile([128, BS], f32, tag="ps", name=f"psh{idx}{c}")
                    nc.tensor.matmul(
                        psh, gr[:, 0, msl], hT[:, 0, :], start=True, stop=False
                    )
                    nc.tensor.matmul(
                        psh, gr[:, 1, msl], hT[:, 1, :], start=False, stop=True
                    )
                    t1 = gwork.tile([128, BS], f32, tag="gt", name=f"t1{idx}{c}")
                    nc.vector.scalar_tensor_tensor(
                        t1, psh, bhh[:, c : c + 1], rs[c], OP.add, OP.mult
                    )
                    t3 = gwork.tile([128, BS], f32, tag="gt", name=f"t3{idx}{c}")
                    nc.vector.tensor_add(t3, psx, t1)
                    cand = gwork.tile([128, BS], f32, tag="gt", name=f"cd{idx}{c}")
                    nc.scalar.activation(
                        cand, t3, AF.Tanh, bias=bxh[:, c : c + 1], scale=1.0
                    )
                    d = gwork.tile([128, BS], f32, tag="gt", name=f"d{idx}{c}")
                    nc.vector.tensor_sub(d, hT[:, c, :], cand)
                    e = gwork.tile([128, BS], f32, tag="gt", name=f"e{idx}{c}")
                    nc.vector.tensor_mul(e, d, zs[c])
                    nc.vector.tensor_add(outT[:, c, :], e, cand)

            gru_cell(0, xinT, h0T, k0_d, rk0_d, bz0, bxh0, bhh0, o0T)
            gru_cell(1, o0T, h1T, k1_d, rk1_d, bz1, bxh1, bhh1, o1T)

            # skip connection
            for c in range(2):
                nc.vector.tensor_add(cT[:, c, :], o1T[:, c, :], xinT[:, c, :])

            # ---------- new_state output ----------
            for bc in range(NBC):
                bsl = slice(bc * 128, (bc + 1) * 128)
                so = bmp.tile([128, 2 * H], f32, tag="so", name=f"so{bc}")
                pt = ptr.tile([128, 2 * H], f32, tag="pt", name=f"ptso{bc}")
                for c in range(2):
                    nc.tensor.transpose(
                        pt[:, c * 128 : (c + 1) * 128],
                        o0T[:, c, bsl].bitcast(f32),
                        ident,
                    )
                    nc.tensor.transpose(
                        pt[:, 256 + c * 128 : 256 + (c + 1) * 128],
                        o1T[:, c, bsl].bitcast(f32),
                        ident,
                    )
                copy_out(so, pt)
                nc.sync.dma_start(stout_d[bsl, :], so)

            # ---------- alpha ----------
            psa = psp.tile([1, BS], f32, tag="ps", name="psa")
            nc.tensor.matmul(psa, fcw[:, 0:1], cT[:, 0, :], start=True, stop=False)
            nc.tensor.matmul(psa, fcw[:, 1:2], cT[:, 1, :], start=False, stop=True)
            al = singles.tile([1, BS], f32, tag="alpha")
            nc.scalar.activation(al, psa, AF.Sigmoid, bias=fcb[0:1, 0:1], scale=1.0)
            nc.sync.dma_start(alpha_d.rearrange("b one -> one b"), al)

            # ---------- conv path ----------
            # x = concat(buf, c0) [BS, KT, F, CH]; processed in batch halves
            # (free dim 256) x feature thirds (4 fb blocks of 128 features).
            buf_f = buf_d.rearrange("b t f c -> b t (f c)")
            c0_f = c0_d.rearrange("b one f c -> b (one f c)")
            bo_f = bufout_d.rearrange("b t f c -> b t (f c)")
            for half in range(NH):
                cbs = []
                for bci in range(2):
                    bc = half * 2 + bci
                    cb = cbp.tile([128, F * OC], f32, tag="cb", name=f"cb{bc}")
                    cbs.append(cb)
                for grp in range(NGRP):
                    fsl = slice(grp * 512, (grp + 1) * 512)
                    xgs = []
                    for bci in range(2):
                        bc = half * 2 + bci
                        bsl = slice(bc * 128, (bc + 1) * 128)
                        xgt = xgp.tile(
                            [128, KT, 512], f32, tag="xg", name=f"xg{bc}_{grp}"
                        )
                        nc.sync.dma_start(xgt[:, : KT - 1, :], buf_f[bsl, :, fsl])
                        nc.sync.dma_start(xgt[:, KT - 1, :], c0_f[bsl, fsl])
                        nc.sync.dma_start(bo_f[bsl, :, fsl], xgt[:, 1:, :])
                        xgs.append(xgt)
                    # transpose 4 fb blocks x 5 t for this half
                    xts = {}
                    for t in range(KT):
                        for fbl in range(4):
                            pt = ptr.tile(
                                [128, 256], f32, tag="pt", name=f"ptx{t}_{fbl}"
                            )
                            for bci in range(2):
                                nc.tensor.transpose(
                                    pt[:, bci * 128 : (bci + 1) * 128],
                                    xgs[bci][:, t, fbl * 128 : (fbl + 1) * 128],
                                    ident,
                                )
                            xt = xtp.tile(
                                [128, 256], f32r, tag="xt", name=f"xt{t}_{fbl}"
                            )
                            copy_out(xt, pt)
                            xts[(t, fbl)] = xt
                    # conv matmuls: 4 psum accumulators over 5 time steps
                    pcs = [
                        psp.tile([80, 256], f32, tag="ps", name=f"pcs{grp}_{j}")
                        for j in range(4)
                    ]
                    for t in range(KT):
                        for j in range(4):
                            nc.tensor.matmul(
                                pcs[j],
                                convl[:, t, :],
                                xts[(t, j)],
                                start=(t == 0),
                                stop=(t == KT - 1),
                            )
                    # drain: pw conv + bn + relu + w_out tanh + add + transpose
                    for j in range(4):
                        fb = grp * 4 + j
                        ysl = wk.tile([80, 256], f32r, tag="ys", name=f"ys{fb}")
                        copy_out(ysl, pcs[j])
                        pp = psp.tile([80, 256], f32, tag="ps", name=f"pp{fb}")
                        nc.tensor.matmul(pp, pwl, ysl, start=True, stop=True)
                        c0o = wk.tile([80, 256], f32, tag="c0o", name=f"c0o{fb}")
                        nc.scalar.activation(
                            c0o, pp, AF.Relu, bias=bnb[:, 0:1], scale=bns[:, 0:1]
                        )
                        hsl = slice(half * 256, (half + 1) * 256)
                        psw = psp.tile([80, 256], f32, tag="ps", name=f"psw{fb}")
                        nc.tensor.matmul(
                            psw,
                            woutl[:, fb, :],
                            cT[:, TAU[fb], hsl],
                            start=True,
                            stop=True,
                        )
                        tht = wk.tile([80, 256], f32, tag="tht", name=f"tht{fb}")
                        nc.scalar.activation(tht, psw, AF.Tanh, scale=1.0)
                        co = wk.tile([80, 256], f32, tag="co", name=f"co{fb}")
                        nc.vector.tensor_add(co, c0o, tht)
                        for bci in range(2):
                            ptc = ptr.tile(
                                [128, 80], f32, tag="pt", name=f"ptc{fb}_{bci}"
                            )
                            nc.tensor.transpose(
                                ptc,
                                co[:, bci * 128 : (bci + 1) * 128],
                                ident[:80, :80],
                            )
                            cbv = cbs[bci].rearrange(
                                "p (o fr t) -> p fr o t", o=O, t=2
                            )
                            copy_out(
                                cbv[:, fb * 8 : (fb + 1) * 8, :, :],
                                ptc.rearrange("p (fr o t) -> p fr o t", o=O, t=2),
                            )
                for bci in range(2):
                    bc = half * 2 + bci
                    bsl = slice(bc * 128, (bc + 1) * 128)
                    nc.sync.dma_start(coefs_d[bsl, :], cbs[bci])

    nc.finalize()
    return nc


_NC_CACHE = None


def _get_nc():
    global _NC_CACHE
    if _NC_CACHE is None:
        _NC_CACHE = _build_nc()
    return _NC_CACHE


def _rne11(x):
    """Round fp32 to float32r (11 explicit mantissa bits, round-nearest-even)."""
    b = np.ascontiguousarray(x, np.float32).view(np.uint32)
    shift = np.uint32(12)  # 23 - 11
    lsb = (b >> shift) & np.uint32(1)
    rounded = (b + np.uint32(0x7FF) + lsb) & np.uint32(0xFFFFF000)
    return rounded.view(np.float32)


def _prep_shared(w_in, gru_k0, gru_rk0, gru_b0, gru_k1, gru_rk1, gru_b1,
                 w_out, fc_a_w, fc_a_b, conv_w, pw_w,
                 bn_gamma, bn_beta, bn_mean, bn_var):
    f = np.float32
    shared = {}

    # dense block-diagonal in-projection lhsT: [2, 128, 128]
    winl = np.zeros((2, 128, 128), f)
    for g in range(G):
        c, gg = divmod(g, 4)
        winl[c, gg * 32 : (gg + 1) * 32, gg * 32 : (gg + 1) * 32] = w_in[g]
    shared["winl"] = _rne11(winl)

    shared["k0"] = _rne11(gru_k0)
    shared["rk0"] = _rne11(gru_rk0)
    shared["k1"] = _rne11(gru_k1)
    shared["rk1"] = _rne11(gru_rk1)

    for nm, b in (("0", gru_b0), ("1", gru_b1)):
        bz = (b[0, : 2 * H] + b[1, : 2 * H]).astype(f)
        shared["bz" + nm] = bz.reshape(4, 128, 1)
        shared["bxh" + nm] = b[0, 2 * H :].astype(f).reshape(2, 128, 1)
        shared["bhh" + nm] = b[1, 2 * H :].astype(f).reshape(2, 128, 1)

    # w_out grouped-linear as per-feature-block lhsT: [12, 128, 80]
    woutl = np.zeros((NFB, 128, 80), f)
    for fb in range(NFB):
        for j in range(80):
            feat = 80 * fb + j
            gw, og = divmod(feat, 120)
            r0 = 32 * gw - 128 * TAU[fb]
            woutl[fb, r0 : r0 + 32, j] = w_out[gw, :, og]
    shared["woutl"] = _rne11(woutl)

    # conv lhsT per time step: [5, 128, 80]
    wg = conv_w.reshape(KT, CH // 2, 2, OC // 2)  # [t, i, g, o]
    convl = np.zeros((KT, 128, 80), f)
    for t in range(KT):
        for fs in range(8):
            for g in range(2):
                for i in range(8):
                    k = fs * 16 + g * 8 + i
                    for o in range(OC // 2):
                        convl[t, k, fs * 10 + g * 5 + o] = wg[t, i, g, o]
    shared["convl"] = _rne11(convl)

    # pointwise conv lhsT: block-diag of pw_w over 8 f values
    pwl = np.zeros((80, 80), f)
    for fs in range(8):
        pwl[fs * 10 : (fs + 1) * 10, fs * 10 : (fs + 1) * 10] = pw_w
    shared["pwl"] = _rne11(pwl)

    inv = 1.0 / np.sqrt(bn_var.astype(np.float64) + BN_EPS)
    s10 = (inv * bn_gamma).astype(f)
    b10 = (bn_beta - bn_mean * inv * bn_gamma).astype(f)
    shared["bns"] = np.tile(s10, 8).reshape(80, 1)
    shared["bnb"] = np.tile(b10, 8).reshape(80, 1)

    shared["fcw"] = _rne11(fc_a_w).reshape(2, 128, 1)
    shared["fcb"] = np.asarray(fc_a_b, f).reshape(1, 1)
    shared["ident"] = np.eye(128, dtype=f)
    return shared


def kernel(emb, c0, df_dec_state, df_convp_buf,
           w_in, gru_k0, gru_rk0, gru_b0, gru_k1, gru_rk1, gru_b1,
           w_out, fc_a_w, fc_a_b, conv_w, pw_w,
           bn_gamma, bn_beta, bn_mean, bn_var):
    nc = _get_nc()
    shared = _prep_shared(
        np.asarray(w_in, np.float32), np.asarray(gru_k0, np.float32),
        np.asarray(gru_rk0, np.float32), np.asarray(gru_b0, np.float32),
        np.asarray(gru_k1, np.float32), np.asarray(gru_rk1, np.float32),
        np.asarray(gru_b1, np.float32), np.asarray(w_out, np.float32),
        np.asarray(fc_a_w, np.float32), np.asarray(fc_a_b, np.float32),
        np.asarray(conv_w, np.float32), np.asarray(pw_w, np.float32),
        np.asarray(bn_gamma, np.float32), np.asarray(bn_beta, np.float32),
        np.asarray(bn_mean, np.float32), np.asarray(bn_var, np.float32),
    )
    emb = np.ascontiguousarray(emb, np.float32)
    c0 = np.ascontiguousarray(c0, np.float32)
    st = np.ascontiguousarray(df_dec_state, np.float32)
    buf = np.ascontiguousarray(df_convp_buf, np.float32)

    in_maps = []
    for i in range(NCORES):
        sl = slice(i * BS, (i + 1) * BS)
        m = dict(shared)
        m["emb"] = emb[sl]
        m["c0"] = c0[sl]
        m["state"] = st[sl]
        m["buf"] = buf[sl]
        in_maps.append(m)

    res = run_bass_kernel_spmd(nc, in_maps, list(range(NCORES))).results

    coefs = np.concatenate([r["coefs"] for r in res], 0).reshape(B, O, 1, F, 2)
    alpha = np.concatenate([r["alpha"] for r in res], 0)
    new_state = np.concatenate([r["state_out"] for r in res], 0)
    new_buf = np.concatenate([r["buf_out"] for r in res], 0)
    return coefs, alpha, new_state, new_buf


# revision 5
# speedup vs baseline: 1.4663x; 1.0675x over previous
"""Trainium2 Bass kernel for nn_DfDecoderStep (GRU decoder step + causal conv).

Data-parallel over batch across 8 NeuronCores (512 rows/core).  On-chip
compute is feature-major (features on SBUF partitions, batch in the free
dim) so every matmul contracts along partitions with weights stationary.
Inputs are transposed on-chip with PE transposes (fp32, exact); matmuls
run in float32r (single-pass, full-rate at free-dim >= 256; ~tf32
precision, rel err ~1e-4).  Grouped linears / grouped conv / pointwise
conv are dense block-diagonal lhsT matrices precomputed on the host.
The conv buffer shift (new_buf) is an exact fp32 copy.
"""

import numpy as np

import concourse.bacc as bacc
import concourse.tile as tile
from concourse import mybir
from concourse.bass_utils import run_bass_kernel_spmd

B, EMB, H, G, F, CH, O, OC, KT = 4096, 256, 256, 8, 96, 16, 5, 10, 5
BN_EPS = 1e-3
NCORES = 8
BS = B // NCORES          # 512 rows per core
NBC = BS // 128           # 4 batch chunks of 128
FCIN = F * CH             # 1536 features per conv frame
NFB = 12                  # conv feature blocks (8 f x 10 (g,o) = 80 outputs each)
NGRP = 3                  # conv fb groups of 4 (PSUM budget)
NH = 2                    # batch halves (free dim 256 for conv-side matmuls)

f32 = mybir.dt.float32
f32r = mybir.dt.float32r
AF = mybir.ActivationFunctionType
OP = mybir.AluOpType

# which 128-wide cT tile each w_out output block contracts against
TAU = [(32 * ((80 * fb) // 120)) // 128 for fb in range(NFB)]


def _build_nc():
    nc = bacc.Bacc("TRN2")

    # ---- I/O ----
    emb_d = nc.dram_tensor("emb", [BS, EMB], f32r, kind="ExternalInput")
    st_d = nc.dram_tensor("state", [BS, 2 * H], f32r, kind="ExternalInput")
    buf_d = nc.dram_tensor("buf", [BS, KT - 1, F, CH], f32r, kind="ExternalInput")
    c0_d = nc.dram_tensor("c0", [BS, 1, F, CH], f32r, kind="ExternalInput")
    winl_d = nc.dram_tensor("winl", [2, 128, 128], f32r, kind="ExternalInput")
    k0_d = nc.dram_tensor("k0", [H, 3 * H], f32r, kind="ExternalInput")
    rk0_d = nc.dram_tensor("rk0", [H, 3 * H], f32r, kind="ExternalInput")
    k1_d = nc.dram_tensor("k1", [H, 3 * H], f32r, kind="ExternalInput")
    rk1_d = nc.dram_tensor("rk1", [H, 3 * H], f32r, kind="ExternalInput")
    bz0_d = nc.dram_tensor("bz0", [4, 128, 1], f32, kind="ExternalInput")
    bxh0_d = nc.dram_tensor("bxh0", [2, 128, 1], f32, kind="ExternalInput")
    bhh0_d = nc.dram_tensor("bhh0", [2, 128, 1], f32, kind="ExternalInput")
    bz1_d = nc.dram_tensor("bz1", [4, 128, 1], f32, kind="ExternalInput")
    bxh1_d = nc.dram_tensor("bxh1", [2, 128, 1], f32, kind="ExternalInput")
    bhh1_d = nc.dram_tensor("bhh1", [2, 128, 1], f32, kind="ExternalInput")
    woutl_d = nc.dram_tensor("woutl", [NFB, 128, 80], f32r, kind="ExternalInput")
    convl_d = nc.dram_tensor("convl", [KT, 128, 80], f32r, kind="ExternalInput")
    pwl_d = nc.dram_tensor("pwl", [80, 80], f32r, kind="ExternalInput")
    bns_d = nc.dram_tensor("bns", [80, 1], f32, kind="ExternalInput")
    bnb_d = nc.dram_tensor("bnb", [80, 1], f32, kind="ExternalInput")
    fcw_d = nc.dram_tensor("fcw", [2, 128, 1], f32r, kind="ExternalInput")
    fcb_d = nc.dram_tensor("fcb", [1, 1], f32, kind="ExternalInput")
    id_d = nc.dram_tensor("ident", [128, 128], f32r, kind="ExternalInput")

    coefs_d = nc.dram_tensor("coefs", [BS, F * OC], f32r, kind="ExternalOutput")
    alpha_d = nc.dram_tensor("alpha", [BS, 1], f32, kind="ExternalOutput")
    stout_d = nc.dram_tensor("state_out", [BS, 2 * H], f32r, kind="ExternalOutput")
    bufout_d = nc.dram_tensor(
        "buf_out", [BS, KT - 1, F, CH], f32r, kind="ExternalOutput"
    )

    with tile.TileContext(nc) as tc:
        with (
            tc.tile_pool(name="singles", bufs=1) as singles,
            tc.tile_pool(name="gwp", bufs=2) as gwp,
            tc.tile_pool(name="featp", bufs=1) as featp,
            tc.tile_pool(name="bm", bufs=4) as bmp,
            tc.tile_pool(name="gwork", bufs=10) as gwork,
            tc.tile_pool(name="xg", bufs=4) as xgp,
            tc.tile_pool(name="xtp", bufs=30) as xtp,
            tc.tile_pool(name="wk", bufs=4) as wk,
            tc.tile_pool(name="cbp", bufs=3) as cbp,
            tc.tile_pool(name="ps", bufs=6, space="PSUM") as psp,
            tc.tile_pool(name="ptr", bufs=2, space="PSUM") as ptr,
        ):
            ncopy = 0

            def copy_out(dst, src):
                # alternate PSUM->SBUF copies between DVE and ACT
                nonlocal ncopy
                ncopy += 1
                if ncopy % 3:
                    nc.vector.tensor_copy(dst, src)
                else:
                    nc.scalar.copy(dst, src)

            # ---------- constants / weights ----------
            ident = singles.tile([128, 128], f32r, tag="ident")
            nc.sync.dma_start(ident, id_d[:, :])
            winl = singles.tile([128, 2, 128], f32r, tag="winl")
            for c in range(2):
                nc.sync.dma_start(winl[:, c, :], winl_d[c])
            woutl = singles.tile([128, NFB, 80], f32r, tag="woutl")
            for j in range(NFB):
                nc.sync.dma_start(woutl[:, j, :], woutl_d[j])
            convl = singles.tile([128, KT, 80], f32r, tag="convl")
            for t in range(KT):
                nc.sync.dma_start(convl[:, t, :], convl_d[t])
            pwl = singles.tile([80, 80], f32r, tag="pwl")
            nc.sync.dma_start(pwl, pwl_d[:, :])
            bns = singles.tile([80, 1], f32, tag="bns")
            nc.sync.dma_start(bns, bns_d[:, :])
            bnb = singles.tile([80, 1], f32, tag="bnb")
            nc.sync.dma_start(bnb, bnb_d[:, :])
            fcw = singles.tile([128, 2], f32r, tag="fcw")
            for c in range(2):
                nc.sync.dma_start(fcw[:, c : c + 1], fcw_d[c])
            fcb = singles.tile([1, 1], f32, tag="fcb")
            nc.sync.dma_start(fcb, fcb_d[:, :])

            def bias_tile(dram, n, tag):
                t = singles.tile([128, n], f32, tag=tag, name=tag)
                for c in range(n):
                    nc.sync.dma_start(t[:, c : c + 1], dram[c])
                return t

            bz0 = bias_tile(bz0_d, 4, "bz0")
            bxh0 = bias_tile(bxh0_d, 2, "bxh0")
            bhh0 = bias_tile(bhh0_d, 2, "bhh0")
            bz1 = bias_tile(bz1_d, 4, "bz1")
            bxh1 = bias_tile(bxh1_d, 2, "bxh1")
            bhh1 = bias_tile(bhh1_d, 2, "bhh1")

            # ---------- persistent feature-major activations (fp32r) ----------
            embT = featp.tile([128, 2, BS], f32r, tag="embT")
            xinT = featp.tile([128, 2, BS], f32r, tag="xinT")
            h0T = featp.tile([128, 2, BS], f32r, tag="h0T")
            h1T = featp.tile([128, 2, BS], f32r, tag="h1T")
            o0T = featp.tile([128, 2, BS], f32r, tag="o0T")
            o1T = featp.tile([128, 2, BS], f32r, tag="o1T")
            cT = featp.tile([128, 2, BS], f32r, tag="cT")

            # ---------- load + transpose emb and state ----------
            ebms = []
            sbms = []
            for bc in range(NBC):
                bsl = slice(bc * 128, (bc + 1) * 128)
                ebm = bmp.tile([128, EMB], f32r, tag="ebm", name=f"ebm{bc}")
                nc.sync.dma_start(ebm, emb_d[bsl, :])
                ebms.append(ebm)
                sbm = bmp.tile([128, 2 * H], f32r, tag="sbm", name=f"sbm{bc}")
                nc.sync.dma_start(sbm, st_d[bsl, :])
                sbms.append(sbm)
            for c in range(2):
                pt = ptr.tile([128, BS], f32r, tag="pt", name=f"ptemb{c}")
                for bc in range(NBC):
                    nc.tensor.transpose(
                        pt[:, bc * 128 : (bc + 1) * 128],
                        ebms[bc][:, c * 128 : (c + 1) * 128],
                        ident,
                    )
                copy_out(embT[:, c, :], pt)
            for tau in range(4):
                pt = ptr.tile([128, BS], f32r, tag="pt", name=f"ptst{tau}")
                for bc in range(NBC):
                    nc.tensor.transpose(
                        pt[:, bc * 128 : (bc + 1) * 128],
                        sbms[bc][:, tau * 128 : (tau + 1) * 128],
                        ident,
                    )
                dst = h0T if tau < 2 else h1T
                copy_out(dst[:, tau % 2, :], pt)

            # ---------- in-projection + relu ----------
            for c in range(2):
                ps = psp.tile([128, BS], f32, tag="ps", name=f"psin{c}")
                nc.tensor.matmul(
                    ps, winl[:, c, :], embT[:, c, :], start=True, stop=True
                )
                nc.vector.tensor_scalar_max(xinT[:, c, :], ps, 0.0)

            # ---------- GRU cells ----------
            def gru_cell(idx, xT, hT, kd, rkd, bz, bxh, bhh, outT):
                gk = gwp.tile([128, 2, 3 * H], f32r, tag="gw", name=f"gk{idx}")
                gr = gwp.tile([128, 2, 3 * H], f32r, tag="gw", name=f"gr{idx}")
                for kc in range(2):
                    nc.sync.dma_start(gk[:, kc, :], kd[kc * 128 : (kc + 1) * 128, :])
                    nc.sync.dma_start(
                        gr[:, kc, :], rkd[kc * 128 : (kc + 1) * 128, :]
                    )
                zs, rs = [], []
                for c in range(4):  # z gates: c 0,1; r gates: c 2,3
                    ps = psp.tile([128, BS], f32, tag="ps", name=f"pszr{idx}{c}")
                    msl = slice(c * 128, (c + 1) * 128)
                    nc.tensor.matmul(
                        ps, gk[:, 0, msl], xT[:, 0, :], start=True, stop=False
                    )
                    nc.tensor.matmul(
                        ps, gk[:, 1, msl], xT[:, 1, :], start=False, stop=False
                    )
                    nc.tensor.matmul(
                        ps, gr[:, 0, msl], hT[:, 0, :], start=False, stop=False
                    )
                    nc.tensor.matmul(
                        ps, gr[:, 1, msl], hT[:, 1, :], start=False, stop=True
                    )
                    g = gwork.tile([128, BS], f32, tag="gt", name=f"g{idx}{c}")
                    nc.scalar.activation(
                        g, ps, AF.Sigmoid, bias=bz[:, c : c + 1], scale=1.0
                    )
                    (zs if c < 2 else rs).append(g)
                for c in range(2):  # candidate + new h
                    msl = slice((4 + c) * 128, (5 + c) * 128)
                    psx = psp.tile([128, BS], f32, tag="ps", name=f"psx{idx}{c}")
                    nc.tensor.matmul(
                        psx, gk[:, 0, msl], xT[:, 0, :], start=True, stop=False
                    )
                    nc.tensor.matmul(
                        psx, gk[:, 1, msl], xT[:, 1, :], start=False, stop=True
                    )
                    psh = psp.tile([128, BS], f32, tag="ps", name=f"psh{idx}{c}")
                    nc.tensor.matmul(
                        psh, gr[:, 0, msl], hT[:, 0, :], start=True, stop=False
                    )
                    nc.tensor.matmul(
                        psh, gr[:, 1, msl], hT[:, 1, :], start=False, stop=True
                    )
                    t1 = gwork.tile([128, BS], f32, tag="gt", name=f"t1{idx}{c}")
                    nc.vector.scalar_tensor_tensor(
                        t1, psh, bhh[:, c : c + 1], rs[c], OP.add, OP.mult
                    )
                    t3 = gwork.tile([128, BS], f32, tag="gt", name=f"t3{idx}{c}")
                    nc.vector.tensor_add(t3, psx, t1)
                    cand = gwork.tile([128, BS], f32, tag="gt", name=f"cd{idx}{c}")
                    nc.scalar.activation(
                        cand, t3, AF.Tanh, bias=bxh[:, c : c + 1], scale=1.0
                    )
                    d = gwork.tile([128, BS], f32, tag="gt", name=f"d{idx}{c}")
                    nc.vector.tensor_sub(d, hT[:, c, :], cand)
                    e = gwork.tile([128, BS], f32, tag="gt", name=f"e{idx}{c}")
                    nc.vector.tensor_mul(e, d, zs[c])
                    nc.vector.tensor_add(outT[:, c, :], e, cand)

            gru_cell(0, xinT, h0T, k0_d, rk0_d, bz0, bxh0, bhh0, o0T)
            gru_cell(1, o0T, h1T, k1_d, rk1_d, bz1, bxh1, bhh1, o1T)

            # skip connection
            for c in range(2):
                nc.vector.tensor_add(cT[:, c, :], o1T[:, c, :], xinT[:, c, :])

            # ---------- new_state output ----------
            for bc in range(NBC):
                bsl = slice(bc * 128, (bc + 1) * 128)
                so = bmp.tile([128, 2 * H], f32r, tag="so", name=f"so{bc}")
                pt = ptr.tile([128, 2 * H], f32r, tag="pt", name=f"ptso{bc}")
                for c in range(2):
                    nc.tensor.transpose(
                        pt[:, c * 128 : (c + 1) * 128],
                        o0T[:, c, bsl],
                        ident,
                    )
                    nc.tensor.transpose(
                        pt[:, 256 + c * 128 : 256 + (c + 1) * 128],
                        o1T[:, c, bsl],
                        ident,
                    )
                copy_out(so, pt)
                nc.sync.dma_start(stout_d[bsl, :], so)

            # ---------- alpha ----------
            psa = psp.tile([1, BS], f32, tag="ps", name="psa")
            nc.tensor.matmul(psa, fcw[:, 0:1], cT[:, 0, :], start=True, stop=False)
            nc.tensor.matmul(psa, fcw[:, 1:2], cT[:, 1, :], start=False, stop=True)
            al = singles.tile([1, BS], f32, tag="alpha")
            nc.scalar.activation(al, psa, AF.Sigmoid, bias=fcb[0:1, 0:1], scale=1.0)
            nc.sync.dma_start(alpha_d.rearrange("b one -> one b"), al)

            # ---------- conv path ----------
            # x = concat(buf, c0) [BS, KT, F, CH]; processed in batch halves
            # (free dim 256) x feature thirds (4 fb blocks of 128 features).
            buf_f = buf_d.rearrange("b t f c -> b t (f c)")
            c0_f = c0_d.rearrange("b one f c -> b (one f c)")
            bo_f = bufout_d.rearrange("b t f c -> b t (f c)")
            for half in range(NH):
                cbs = []
                for bci in range(2):
                    bc = half * 2 + bci
                    cb = cbp.tile([128, F * OC], f32r, tag="cb", name=f"cb{bc}")
                    cbs.append(cb)
                for grp in range(NGRP):
                    fsl = slice(grp * 512, (grp + 1) * 512)
                    xgs = []
                    for bci in range(2):
                        bc = half * 2 + bci
                        bsl = slice(bc * 128, (bc + 1) * 128)
                        xgt = xgp.tile(
                            [128, KT, 512], f32r, tag="xg", name=f"xg{bc}_{grp}"
                        )
                        nc.sync.dma_start(xgt[:, : KT - 1, :], buf_f[bsl, :, fsl])
                        nc.sync.dma_start(xgt[:, KT - 1, :], c0_f[bsl, fsl])
                        nc.sync.dma_start(bo_f[bsl, :, fsl], xgt[:, 1:, :])
                        xgs.append(xgt)
                    # transpose 4 fb blocks x 5 t for this half
                    xts = {}
                    for t in range(KT):
                        for fbl in range(4):
                            pt = ptr.tile([128, 256], f32r, tag="pt", name=f"ptx{t}_{fbl}"
                            )
                            for bci in range(2):
                                nc.tensor.transpose(
                                    pt[:, bci * 128 : (bci + 1) * 128],
                                    xgs[bci][:, t, fbl * 128 : (fbl + 1) * 128],
                                    ident,
                                )
                            xt = xtp.tile(
                                [128, 256], f32r, tag="xt", name=f"xt{t}_{fbl}"
                            )
                            copy_out(xt, pt)
                            xts[(t, fbl)] = xt
                    # conv matmuls: 4 psum accumulators over 5 time steps
                    pcs = [
                        psp.tile([80, 256], f32, tag="ps", name=f"pcs{grp}_{j}")
                        for j in range(4)
                    ]
                    for t in range(KT):
                        for j in range(4):
                            nc.tensor.matmul(
                                pcs[j],
                                convl[:, t, :],
                                xts[(t, j)],
                                start=(t == 0),
                                stop=(t == KT - 1),
                            )
                    # drain: pw conv + bn + relu + w_out tanh + add + transpose
                    for j in range(4):
                        fb = grp * 4 + j
                        ysl = wk.tile([80, 256], f32r, tag="ys", name=f"ys{fb}")
                        copy_out(ysl, pcs[j])
                        pp = psp.tile([80, 256], f32, tag="ps", name=f"pp{fb}")
                        nc.tensor.matmul(pp, pwl, ysl, start=True, stop=True)
                        c0o = wk.tile([80, 256], f32, tag="c0o", name=f"c0o{fb}")
                        nc.scalar.activation(
                            c0o, pp, AF.Relu, bias=bnb[:, 0:1], scale=bns[:, 0:1]
                        )
                        hsl = slice(half * 256, (half + 1) * 256)
                        psw = psp.tile([80, 256], f32, tag="ps", name=f"psw{fb}")
                        nc.tensor.matmul(
                            psw,
                            woutl[:, fb, :],
                            cT[:, TAU[fb], hsl],
                            start=True,
                            stop=True,
                        )
                        tht = wk.tile([80, 256], f32, tag="tht", name=f"tht{fb}")
                        nc.scalar.activation(tht, psw, AF.Tanh, scale=1.0)
                        co = wk.tile([80, 256], f32r, tag="co", name=f"co{fb}")
                        nc.vector.tensor_add(co, c0o, tht)
                        for bci in range(2):
                            ptc = ptr.tile([128, 80], f32r, tag="pt", name=f"ptc{fb}_{bci}"
                            )
                            nc.tensor.transpose(
                                ptc,
                                co[:, bci * 128 : (bci + 1) * 128],
                                ident[:80, :80],
                            )
                            cbv = cbs[bci].rearrange(
                                "p (o fr t) -> p fr o t", o=O, t=2
                            )
                            copy_out(
                                cbv[:, fb * 8 : (fb + 1) * 8, :, :],
                                ptc.rearrange("p (fr o t) -> p fr o t", o=O, t=2),
                            )
                for bci in range(2):
                    bc = half * 2 + bci
                    bsl = slice(bc * 128, (bc + 1) * 128)
                    nc.sync.dma_start(coefs_d[bsl, :], cbs[bci])

    nc.finalize()
    return nc


_NC_CACHE = None


def _get_nc():
    global _NC_CACHE
    if _NC_CACHE is None:
        _NC_CACHE = _build_nc()
    return _NC_CACHE


def _rne11(x):
    """Round fp32 to float32r (11 explicit mantissa bits, round-nearest-even)."""
    b = np.ascontiguousarray(x, np.float32).view(np.uint32)
    shift = np.uint32(12)  # 23 - 11
    lsb = (b >> shift) & np.uint32(1)
    rounded = (b + np.uint32(0x7FF) + lsb) & np.uint32(0xFFFFF000)
    return rounded.view(np.float32)


def _prep_shared(w_in, gru_k0, gru_rk0, gru_b0, gru_k1, gru_rk1, gru_b1,
                 w_out, fc_a_w, fc_a_b, conv_w, pw_w,
                 bn_gamma, bn_beta, bn_mean, bn_var):
    f = np.float32
    shared = {}

    # dense block-diagonal in-projection lhsT: [2, 128, 128]
    winl = np.zeros((2, 128, 128), f)
    for g in range(G):
        c, gg = divmod(g, 4)
        winl[c, gg * 32 : (gg + 1) * 32, gg * 32 : (gg + 1) * 32] = w_in[g]
    shared["winl"] = _rne11(winl)

    shared["k0"] = _rne11(gru_k0)
    shared["rk0"] = _rne11(gru_rk0)
    shared["k1"] = _rne11(gru_k1)
    shared["rk1"] = _rne11(gru_rk1)

    for nm, b in (("0", gru_b0), ("1", gru_b1)):
        bz = (b[0, : 2 * H] + b[1, : 2 * H]).astype(f)
        shared["bz" + nm] = bz.reshape(4, 128, 1)
        shared["bxh" + nm] = b[0, 2 * H :].astype(f).reshape(2, 128, 1)
        shared["bhh" + nm] = b[1, 2 * H :].astype(f).reshape(2, 128, 1)

    # w_out grouped-linear as per-feature-block lhsT: [12, 128, 80]
    woutl = np.zeros((NFB, 128, 80), f)
    for fb in range(NFB):
        for j in range(80):
            feat = 80 * fb + j
            gw, og = divmod(feat, 120)
            r0 = 32 * gw - 128 * TAU[fb]
            woutl[fb, r0 : r0 + 32, j] = w_out[gw, :, og]
    shared["woutl"] = _rne11(woutl)

    # conv lhsT per time step: [5, 128, 80]
    wg = conv_w.reshape(KT, CH // 2, 2, OC // 2)  # [t, i, g, o]
    convl = np.zeros((KT, 128, 80), f)
    for t in range(KT):
        for fs in range(8):
            for g in range(2):
                for i in range(8):
                    k = fs * 16 + g * 8 + i
                    for o in range(OC // 2):
                        convl[t, k, fs * 10 + g * 5 + o] = wg[t, i, g, o]
    shared["convl"] = _rne11(convl)

    # pointwise conv lhsT: block-diag of pw_w over 8 f values
    pwl = np.zeros((80, 80), f)
    for fs in range(8):
        pwl[fs * 10 : (fs + 1) * 10, fs * 10 : (fs + 1) * 10] = pw_w
    shared["pwl"] = _rne11(pwl)

    inv = 1.0 / np.sqrt(bn_var.astype(np.float64) + BN_EPS)
    s10 = (inv * bn_gamma).astype(f)
    b10 = (bn_beta - bn_mean * inv * bn_gamma).astype(f)
    shared["bns"] = np.tile(s10, 8).reshape(80, 1)
    shared["bnb"] = np.tile(b10, 8).reshape(80, 1)

    shared["fcw"] = _rne11(fc_a_w).reshape(2, 128, 1)
    shared["fcb"] = np.asarray(fc_a_b, f).reshape(1, 1)
    shared["ident"] = np.eye(128, dtype=f)
    return shared


def kernel(emb, c0, df_dec_state, df_convp_buf,
           w_in, gru_k0, gru_rk0, gru_b0, gru_k1, gru_rk1, gru_b1,
           w_out, fc_a_w, fc_a_b, conv_w, pw_w,
           bn_gamma, bn_beta, bn_mean, bn_var):
    nc = _get_nc()
    shared = _prep_shared(
        np.asarray(w_in, np.float32), np.asarray(gru_k0, np.float32),
        np.asarray(gru_rk0, np.float32), np.asarray(gru_b0, np.float32),
        np.asarray(gru_k1, np.float32), np.asarray(gru_rk1, np.float32),
        np.asarray(gru_b1, np.float32), np.asarray(w_out, np.float32),
        np.asarray(fc_a_w, np.float32), np.asarray(fc_a_b, np.float32),
        np.asarray(conv_w, np.float32), np.asarray(pw_w, np.float32),
        np.asarray(bn_gamma, np.float32), np.asarray(bn_beta, np.float32),
        np.asarray(bn_mean, np.float32), np.asarray(bn_var, np.float32),
    )
    emb = np.ascontiguousarray(emb, np.float32)
    c0 = np.ascontiguousarray(c0, np.float32)
    st = np.ascontiguousarray(df_dec_state, np.float32)
    buf = np.ascontiguousarray(df_convp_buf, np.float32)

    in_maps = []
    for i in range(NCORES):
        sl = slice(i * BS, (i + 1) * BS)
        m = dict(shared)
        m["emb"] = emb[sl]
        m["c0"] = c0[sl]
        m["state"] = st[sl]
        m["buf"] = buf[sl]
        in_maps.append(m)

    res = run_bass_kernel_spmd(nc, in_maps, list(range(NCORES))).results

    coefs = np.concatenate([r["coefs"] for r in res], 0).reshape(B, O, 1, F, 2)
    alpha = np.concatenate([r["alpha"] for r in res], 0)
    new_state = np.concatenate([r["state_out"] for r in res], 0)
    new_buf = np.concatenate([r["buf_out"] for r in res], 0)
    return coefs, alpha, new_state, new_buf
